# revision 2
# baseline (speedup 1.0000x reference)
"""HMM window log-likelihood on 8 NeuronCores (data-parallel over batch).

Math: reference computes, per batch column b,
    y[b] = exp(logsumexp_i x_T[b,i]),  x via log-space forward recursion.
Equivalently in linear space with row-normalized transition matrices
W_t = exp(w[t-1]) / rowsum, emission table L = softmax(distros, axis=1):
    y[b] = 1^T diag(em_T) W_T ... diag(em_1) W_1 em_0
We evaluate it as a BACKWARD recursion (avoids transposing W on device):
    beta_L = 1;  beta_{t-1} = W_t^T (em_t . beta_t)
    y[b] = sum_i em_0[i,b] beta_0[i,b]
with per-step rescale factors g_t (host-computed from column 0, f64) folded
into recipSg[:,t] = g_t / rowsum_t to keep everything in bf16/f32 range.
em_t[i,b] = L[i, bin(b,t)] is computed on the PE as dLT^T @ G_t where
dL[i,k] = L[i,k]-L[i,k-1] and G_t[k,b] = [bin(b,t) >= k] (0/1 indicators).
Device returns colsum[b] = y[b] * prod(g); host: lnY = log(colsum)+C, y=exp.
The true lnY is ~ -584.6 for these inputs, so y underflows f32 to 0.0 —
exactly matching the reference (which also underflows in f32).
"""
import sys, os
for p in ("/opt/trn_rl_repo",):
    if p not in sys.path:
        sys.path.insert(0, p)
import numpy as np
import ml_dtypes

from concourse import bass, bacc, mybir
from concourse.tile import TileContext
from concourse.bass_utils import run_bass_kernel_spmd

W, L, B, NB = 128, 256, 4096, 10
NCORES = 8
BC = B // NCORES          # 512 batch cols per core
BH = BC // 2              # two half-chains of 256
TBLK = 16                 # G streaming block (t's per DMA)

LAST_LNY = None           # debug: device-derived lnY per batch col
LAST_RESULTS = None       # debug: raw BassKernelResults

_CACHED = None            # (nc,) build cache


def _build_nc():
    nc = bacc.Bacc("TRN2", target_bir_lowering=False, debug=False,
                   num_devices=NCORES)
    bf16, f32 = mybir.dt.bfloat16, mybir.dt.float32

    wt = nc.dram_tensor("wt", [W, L - 1, W], bf16, kind="ExternalInput")
    dlt = nc.dram_tensor("dlt", [NB, W], bf16, kind="ExternalInput")
    rsg = nc.dram_tensor("rsg", [W, L], f32, kind="ExternalInput")
    g10 = nc.dram_tensor("g10", [NB, L, BC], bf16, kind="ExternalInput")
    ones = nc.dram_tensor("ones", [W, 1], bf16, kind="ExternalInput")
    colsum = nc.dram_tensor("colsum", [1, BC], f32, kind="ExternalOutput")

    Copy = mybir.ActivationFunctionType.Copy

    with TileContext(nc) as tc:
        with tc.sbuf_pool(name="sb", bufs=2) as sb, \
                tc.psum_pool(name="ps", bufs=2) as ps:
            dlt_sb = sb.tile([NB, W], bf16, bufs=1)
            nc.sync.dma_start(dlt_sb, dlt.ap())
            rsg_sb = sb.tile([W, L], f32, bufs=1)
            nc.sync.dma_start(rsg_sb, rsg.ap())
            ones_sb = sb.tile([W, 1], bf16, bufs=1)
            nc.sync.dma_start(ones_sb, ones.ap())

            # all 255 transition matrices resident; chunked DMAs in backward
            # order so the scan can start as soon as the tail chunk lands
            wt_sb = sb.tile([W, L - 1, W], bf16, bufs=1)
            for cc in range((L - 1 + 7) // 8 - 1, -1, -1):
                t0 = cc * 8
                cnt = min(8, L - 1 - t0)
                nc.sync.dma_start(wt_sb[:, t0:t0 + cnt, :],
                                  wt.ap()[:, t0:t0 + cnt, :])

            cs_ps = None
            beta_ps = [None, None]
            for blk in range(L // TBLK - 1, -1, -1):
                g_sb = sb.tile([NB, TBLK, BC], bf16, tag="g", bufs=3)
                nc.sync.dma_start(
                    g_sb, g10.ap()[:, blk * TBLK:(blk + 1) * TBLK, :])
                for ti in range(TBLK - 1, -1, -1):
                    t = blk * TBLK + ti
                    for h in (0, 1):
                        em_ps = ps.tile([W, BH], f32, tag=f"em{h}", bufs=2)
                        nc.tensor.matmul(
                            em_ps, dlt_sb,
                            g_sb[:, ti, h * BH:(h + 1) * BH],
                            start=True, stop=True)
                        em_sb = sb.tile([W, BH], bf16, tag=f"emsb{h}", bufs=3)
                        nc.scalar.activation(em_sb, em_ps, Copy,
                                             scale=rsg_sb[:, t:t + 1])
                        if t == L - 1:
                            c_sb = em_sb
                        else:
                            c_sb = sb.tile([W, BH], bf16, tag=f"c{h}", bufs=3)
                            nc.vector.tensor_mul(c_sb, beta_ps[h], em_sb)
                        if t > 0:
                            b_ps = ps.tile([W, BH], f32, tag=f"b{h}", bufs=2)
                            nc.tensor.matmul(b_ps, wt_sb[:, t - 1, :], c_sb,
                                             start=True, stop=True)
                            beta_ps[h] = b_ps
                        else:
                            if cs_ps is None:
                                cs_ps = ps.tile([1, BC], f32, tag="em0",
                                                bufs=2)
                            nc.tensor.matmul(cs_ps[:, h * BH:(h + 1) * BH],
                                             ones_sb, c_sb,
                                             start=True, stop=True)

            cs_sb = sb.tile([1, BC], f32, bufs=1)
            nc.vector.tensor_copy(cs_sb, cs_ps)
            nc.sync.dma_start(colsum.ap(), cs_sb)
    nc.compile()
    return nc


def _host_prep(data, input_distros, dense_layer_weights):
    f64 = np.float64
    we = np.exp(dense_layer_weights.astype(f64))           # (255,W,W)
    rowsum = we.sum(axis=2)                                # (255,W)
    recip = 1.0 / rowsum
    d = input_distros.astype(f64)
    d = d - d.max(axis=1, keepdims=True)
    e = np.exp(d)
    Ll = e / e.sum(axis=1, keepdims=True)                  # (W,NB) softmax rows
    # bins exactly as reference: floor(v / 0.1) in f32
    bins = np.minimum(NB - 1, np.floor(
        data / np.float32(0.1)).astype(np.int32))          # (B,L)

    # column-0 f64 backward pass -> per-step rescale g_t, offset C
    beta = np.ones(W, dtype=f64)
    Cacc = 0.0
    g = np.ones(L, dtype=f64)
    for t in range(L - 1, 0, -1):
        c = Ll[np.arange(W), bins[0, t]] * beta * recip[t - 1]
        tmp = we[t - 1].T @ c
        f = tmp.max()
        g[t] = 1.0 / f
        Cacc += np.log(f)
        beta = tmp * g[t]

    rsg = np.ones((W, L), dtype=np.float32)
    rsg[:, 1:] = (recip.T * g[None, 1:]).astype(np.float32)

    dL = Ll.copy()
    dL[:, 1:] -= Ll[:, :-1]
    dlt = np.ascontiguousarray(dL.T).astype(ml_dtypes.bfloat16)  # (NB,W)

    wt = np.ascontiguousarray(
        we.transpose(1, 0, 2)).astype(ml_dtypes.bfloat16)  # (W,255,W)

    # G[k,t,b] = [bins[b,t] >= k]   (G[0] == 1)
    g10 = (bins.T[None, :, :] >= np.arange(NB)[:, None, None]
           ).astype(ml_dtypes.bfloat16)                    # (NB,L,B)
    ones_v = np.ones((W, 1), dtype=ml_dtypes.bfloat16)
    return wt, dlt, rsg, g10, ones_v, Cacc


def kernel(data, input_distros, dense_layer_weights):
    global LAST_LNY, LAST_RESULTS, _CACHED
    wt, dlt, rsg, g10, ones_v, Cacc = _host_prep(
        np.asarray(data), np.asarray(input_distros),
        np.asarray(dense_layer_weights))

    if _CACHED is None:
        _CACHED = _build_nc()
    nc = _CACHED

    in_maps = []
    for c in range(NCORES):
        in_maps.append({
            "wt": wt, "dlt": dlt, "rsg": rsg, "ones": ones_v,
            "g10": np.ascontiguousarray(g10[:, :, c * BC:(c + 1) * BC]),
        })
    res = run_bass_kernel_spmd(
        nc, in_maps, core_ids=list(range(NCORES)),
        trace=bool(int(os.environ.get("KERNEL_TRACE", "0"))),
        tmpdir=os.environ.get("KERNEL_TRACE_DIR") or None)
    LAST_RESULTS = res
    cs = np.concatenate([res.results[c]["colsum"].reshape(-1)
                         for c in range(NCORES)])           # (B,)
    lnY = np.log(cs.astype(np.float64)) + Cacc
    LAST_LNY = lnY
    y = np.exp(lnY).astype(np.float32).reshape(B, 1)
    return y



# revision 9
# speedup vs baseline: 1.3751x; 1.3751x over previous
"""HMM window log-likelihood on 8 NeuronCores (data-parallel over batch).

Math: reference computes, per batch column b,
    y[b] = exp(logsumexp_i x_T[b,i]),  x via log-space forward recursion.
Equivalently in linear space with row-normalized transition matrices
W_t = exp(w[t-1]) / rowsum, emission table L = softmax(distros, axis=1):
    y[b] = 1^T diag(em_T) W_T ... diag(em_1) W_1 em_0
Evaluated as a BACKWARD recursion (avoids transposing W on device):
    beta_L = 1;  beta_{t-1} = W_t'^T (em_t . beta_t)
    y[b] = sum_i em_0[i,b] beta_0[i,b]
where W_t' carries the row-normalization AND per-step rescale factors g_t
(host-computed from column 0 in f64) folded into its rows. Per step the
device does:
    em(t)  = dlt^T G_t            (PE matmul, K=10 indicator trick)
    em_sb  = copy(em)             (ACT drain PSUM->SBUF bf16, off-chain)
    c(t)   = em_sb . beta(t)      (DVE, one PSUM operand; two 256-col chains)
    beta() = wt'[t-1]^T c(t)      (PE matmul)
em[i,b] = L[i, bin(b,t)] via dL[i,k] = L[i,k]-L[i,k-1] against cumulative
indicators G[k,b] = [bin(b,t) >= k].
Device returns colsum[b] = y[b] * prod(g); host: lnY = log(colsum)+C.
The true lnY is ~ -584.6 for these inputs, so y underflows f32 to 0.0 —
exactly matching the reference (which also underflows in f32).
"""
import sys, os
for p in ("/opt/trn_rl_repo",):
    if p not in sys.path:
        sys.path.insert(0, p)
import numpy as np
import ml_dtypes

from concourse import bass, bacc, mybir
from concourse.tile import TileContext
from concourse.bass_utils import run_bass_kernel_spmd

W, L, B, NB = 128, 256, 4096, 10
NCORES = 8
BC = B // NCORES          # 512 batch cols per core
BH = BC // 2              # two chains of 256 cols
TBLK = 16                 # G streaming block (t's per DMA)
EMLOOK = 3                # em matmul lookahead (PSUM tiles)

LAST_LNY = None           # debug: device-derived lnY per batch col
LAST_RESULTS = None       # debug: raw BassKernelResults

_CACHED = None            # (nc,) build cache


def _build_nc():
    nc = bacc.Bacc("TRN2", target_bir_lowering=False, debug=False,
                   num_devices=NCORES)
    bf16, f32 = mybir.dt.bfloat16, mybir.dt.float32
    Copy = mybir.ActivationFunctionType.Copy

    wt = nc.dram_tensor("wt", [W, L - 1, W], bf16, kind="ExternalInput")
    dlt = nc.dram_tensor("dlt", [NB, W], bf16, kind="ExternalInput")
    g10 = nc.dram_tensor("g10", [NB, L, BC], bf16, kind="ExternalInput")
    ones = nc.dram_tensor("ones", [W, 1], bf16, kind="ExternalInput")
    colsum = nc.dram_tensor("colsum", [1, BC], f32, kind="ExternalOutput")

    with TileContext(nc) as tc:
        with tc.sbuf_pool(name="sb", bufs=2) as sb, \
                tc.psum_pool(name="ps", bufs=2) as ps:
            dlt_sb = sb.tile([NB, W], bf16, bufs=1)
            nc.sync.dma_start(dlt_sb, dlt.ap())
            ones_sb = sb.tile([W, 1], bf16, bufs=1)
            nc.sync.dma_start(ones_sb, ones.ap())

            # all 255 transition matrices resident; chunked DMAs in backward
            # order so the scan can start as soon as the tail chunk lands
            wt_sb = sb.tile([W, L - 1, W], bf16, bufs=1)
            for cc in range((L - 1 + 7) // 8 - 1, -1, -1):
                t0 = cc * 8
                cnt = min(8, L - 1 - t0)
                nc.sync.dma_start(wt_sb[:, t0:t0 + cnt, :],
                                  wt.ap()[:, t0:t0 + cnt, :])

            g_tiles = {}

            def ensure_g(blk):
                if 0 <= blk < L // TBLK and blk not in g_tiles:
                    gt = sb.tile([NB, TBLK, BC], bf16, tag="g", bufs=3)
                    nc.sync.dma_start(
                        gt, g10.ap()[:, blk * TBLK:(blk + 1) * TBLK, :])
                    g_tiles[blk] = gt

            def em_matmul(t):
                blk, ti = t // TBLK, t % TBLK
                ensure_g(blk)
                ensure_g(blk - 1)       # prefetch next block (scan backward)
                e = ps.tile([W, BC], f32, tag="em", bufs=EMLOOK)
                nc.tensor.matmul(e, dlt_sb, g_tiles[blk][:, ti, :],
                                 start=True, stop=True)
                return e

            def em_drain(e):
                es = sb.tile([W, BC], bf16, tag="emsb", bufs=EMLOOK)
                nc.scalar.activation(es, e, Copy)
                return es

            em_q = [em_matmul(L - 1 - i) for i in range(EMLOOK)]
            es_q = [em_drain(em_q.pop(0)) for _ in range(2)]

            beta_ps = [None, None]
            cs_ps = None
            for t in range(L - 1, -1, -1):
                em_sb = es_q.pop(0)
                c_sb = [None, None]
                for h in (0, 1):
                    lo = h * BH
                    if t == L - 1:
                        c_sb[h] = em_sb[:, lo:lo + BH]
                    else:
                        c = sb.tile([W, BH], bf16, tag=f"c{h}", bufs=3)
                        nc.vector.tensor_mul(c, em_sb[:, lo:lo + BH],
                                             beta_ps[h])
                        c_sb[h] = c
                if t > 0:
                    for h in (0, 1):
                        b = ps.tile([W, BH], f32, tag=f"b{h}", bufs=2)
                        nc.tensor.matmul(b, wt_sb[:, t - 1, :], c_sb[h],
                                         start=True, stop=True)
                        beta_ps[h] = b
                else:
                    cs_ps = ps.tile([1, BC], f32, tag="cs", bufs=1)
                    for h in (0, 1):
                        nc.tensor.matmul(cs_ps[:, h * BH:(h + 1) * BH],
                                         ones_sb, c_sb[h],
                                         start=True, stop=True)
                # refill lookahead queues (emitted after this t's chain ops
                # so the PE/ACT queues interleave chain work with lookahead)
                if t - EMLOOK >= 0:
                    em_q.append(em_matmul(t - EMLOOK))
                if t - 2 >= 0:
                    es_q.append(em_drain(em_q.pop(0)))

            cs_sb = sb.tile([1, BC], f32, bufs=1)
            nc.vector.tensor_copy(cs_sb, cs_ps)
            nc.sync.dma_start(colsum.ap(), cs_sb)
    nc.compile()
    return nc


def _host_prep(data, input_distros, dense_layer_weights):
    f64 = np.float64
    we = np.exp(dense_layer_weights.astype(f64))           # (255,W,W)
    rowsum = we.sum(axis=2)                                # (255,W)
    recip = 1.0 / rowsum
    d = input_distros.astype(f64)
    d = d - d.max(axis=1, keepdims=True)
    e = np.exp(d)
    Ll = e / e.sum(axis=1, keepdims=True)                  # (W,NB) softmax rows
    # bins exactly as reference: floor(v / 0.1) in f32
    bins = np.minimum(NB - 1, np.floor(
        data / np.float32(0.1)).astype(np.int32))          # (B,L)

    # column-0 f64 backward pass -> per-step rescale g_t, offset C
    beta = np.ones(W, dtype=f64)
    Cacc = 0.0
    g = np.ones(L, dtype=f64)
    for t in range(L - 1, 0, -1):
        c = Ll[np.arange(W), bins[0, t]] * beta * recip[t - 1]
        tmp = we[t - 1].T @ c
        f = tmp.max()
        g[t] = 1.0 / f
        Cacc += np.log(f)
        beta = tmp * g[t]

    # row-normalization (recip) and per-step rescale (g) folded into the
    # transition weights: device matmul contracts partition k, so
    # wt'[k, t-1, i] = we[t-1, k, i] * recip[t-1, k] * g[t]
    rsg = recip.T * g[None, 1:]                            # (W, L-1)
    wtp = np.ascontiguousarray(we.transpose(1, 0, 2))      # [k,t-1,i]=we[t-1,k,i]
    wtp *= rsg[:, :, None]
    wt = wtp.astype(ml_dtypes.bfloat16)                    # (W, 255, W)

    dL = Ll.copy()
    dL[:, 1:] -= Ll[:, :-1]
    dlt = np.ascontiguousarray(dL.T).astype(ml_dtypes.bfloat16)  # (NB,W)

    # G[k,t,b] = [bins[b,t] >= k]   (G[0] == 1)
    g10 = (bins.T[None, :, :] >= np.arange(NB)[:, None, None]
           ).astype(ml_dtypes.bfloat16)                    # (NB,L,B)
    ones_v = np.ones((W, 1), dtype=ml_dtypes.bfloat16)
    return wt, dlt, g10, ones_v, Cacc


def kernel(data, input_distros, dense_layer_weights):
    global LAST_LNY, LAST_RESULTS, _CACHED
    wt, dlt, g10, ones_v, Cacc = _host_prep(
        np.asarray(data), np.asarray(input_distros),
        np.asarray(dense_layer_weights))

    if _CACHED is None:
        _CACHED = _build_nc()
    nc = _CACHED

    in_maps = []
    for c in range(NCORES):
        in_maps.append({
            "wt": wt, "dlt": dlt, "ones": ones_v,
            "g10": np.ascontiguousarray(g10[:, :, c * BC:(c + 1) * BC]),
        })
    res = run_bass_kernel_spmd(
        nc, in_maps, core_ids=list(range(NCORES)),
        trace=bool(int(os.environ.get("KERNEL_TRACE", "0"))),
        tmpdir=os.environ.get("KERNEL_TRACE_DIR") or None)
    LAST_RESULTS = res
    cs = np.concatenate([res.results[c]["colsum"].reshape(-1)
                         for c in range(NCORES)])           # (B,)
    lnY = np.log(cs.astype(np.float64)) + Cacc
    LAST_LNY = lnY
    y = np.exp(lnY).astype(np.float32).reshape(B, 1)
    return y
